# revision 51
# baseline (speedup 1.0000x reference)
"""GCN (6-layer GCNConv) Trainium2 Bass kernel — v8.

Data-parallel over batch (1 mesh per NeuronCore).

Algebraic restructuring vs v6 (1.457 ms):
  - Layer pairs (1,2), (3,4), (5,6) have no ReLU inside, so
    A_hat(A_hat(x Wa) Wb) = A_hat^2(x) (Wa Wb) with host-premultiplied
    W12/W34/W56. Biases become rank-1 terms (A_hat 1 = s rows).
  - The img-feature half of the input is rank-1 across nodes:
    A_hat^2([V | 1 (x) f]) = [A_hat^2 V | s2 (x) f]. So phases A,B apply
    A_hat to WIDTH-3 vertex data; only C,D are 512-wide; E,F width-3.
  - Symmetric norm factorizes: tables prescaled by dinv, one-hot is pure
    0/1 (is_equal only, no norm multiply), post-scale dinv/dinv^2 folded
    into the PSUM-drain activation. Self-loop = identity matmul into the
    same PSUM accumulation (start=True slot).
  - Gathers carry 8 128-slot chunks (1024 idxs, the HW num_idxs encode
    limit; 1152+ crashes the Q7); tiles straddle gather boundaries.
    61 gathers/phase instead of 486 total amortizes the ~1us SWDGE fixed
    cost. Pool serializes each gather's desc-gen + transfer, so the whole
    kernel rides the Pool timeline (~1.1 ms floor at 256B/1KB rows).
  - Deep pools (10 gather/one-hot bufs narrow, 7 wide) keep the Pool fed;
    descriptor-ring scratch shrunk to 16KB to make SBUF room.
  - prepare_only+trigger_dma measured 3x WORSE here (8us/prep spans +
    1.4us InstIncSwdgeSem) - do not revisit without a runtime fix.
  - Phase A needs no device gathers at all: its gather source (dinv*V)
    and the one-hot tables are host-known, so the edge-ordered operands
    (gA) and phase-A one-hots (ohA) are uploaded pre-laid-out; phase A
    runs off Pool entirely (~60us saved).
"""
import sys
import time
import os

sys.path.insert(0, "/opt/trn_rl_repo")
import numpy as np
import ml_dtypes
from contextlib import ExitStack

import concourse.bass as bass
import concourse.bacc as bacc
import concourse.mybir as mybir
import concourse.tile as tile
from concourse.bass_utils import run_bass_kernel_spmd
from concourse.masks import make_identity

P = 128
F32 = mybir.dt.float32
BF16 = mybir.dt.bfloat16
I16 = mybir.dt.int16
BF = ml_dtypes.bfloat16

FM = 512   # hidden width
KM = FM // P
FN = 128   # narrow table row width in elems (256B min gather row)
NW = 4     # useful narrow cols (3 data + pad)


def _pack_graph(src, dst, N):
    """Relabel nodes into degree-balanced 128-node tiles (cap C*P edges per
    tile). slot[t, c*P+p] = dst offset in tile (or -1 pad), idx = src node."""
    T = (N + P - 1) // P
    NP = T * P
    E = len(src)
    indeg = np.bincount(dst, minlength=N)
    C = max(1, int(np.ceil(E / (T * P))))

    order = np.argsort(-indeg, kind="stable")
    while True:
        cap = C * P
        load = np.zeros(T, np.int64)
        count = np.zeros(T, np.int64)
        assign = np.empty(N, np.int64)
        ok = True
        for v in order:
            d = int(indeg[v])
            best_t, best_rem = -1, -1
            for t in range(T):
                if count[t] < P:
                    rem = cap - load[t]
                    if rem > best_rem:
                        best_rem, best_t = rem, t
            if best_t < 0 or load[best_t] + d > cap:
                ok = False
                break
            assign[v] = best_t
            load[best_t] += d
            count[best_t] += 1
        if ok:
            break
        C += 1

    perm = np.full(NP, -1, np.int64)
    new_of_old = np.empty(N, np.int64)
    cursor = np.zeros(T, np.int64)
    for v in range(N):
        t = assign[v]
        nid = t * P + cursor[t]
        cursor[t] += 1
        perm[nid] = v
        new_of_old[v] = nid

    deg = (indeg + 1).astype(np.float64)          # +1 self-loop (GCN)
    dinv = 1.0 / np.sqrt(deg)

    src_n = new_of_old[src]
    dst_n = new_of_old[dst]
    tile_of_e = dst_n // P
    order_e = np.argsort(tile_of_e, kind="stable")
    src_n, dst_n = src_n[order_e], dst_n[order_e]
    tile_of_e = tile_of_e[order_e]

    CP = C * P
    gsrc = np.zeros((T, CP), np.int32)
    slot = np.full((T, CP), -1.0, np.float32)
    starts = np.searchsorted(tile_of_e, np.arange(T + 1))
    for t in range(T):
        lo, hi = starts[t], starts[t + 1]
        n_e = hi - lo
        assert n_e <= CP, (t, n_e, CP)
        gsrc[t, :n_e] = src_n[lo:hi]
        slot[t, :n_e] = (dst_n[lo:hi] - t * P).astype(np.float32)

    # int16 index table: wrapped [i%16, i//16], replicated to all 8 stripes
    SW = CP // 16
    idx16 = np.zeros((P, T * SW), np.int16)
    for t in range(T):
        flat = gsrc[t].astype(np.int16)
        idx16[:, t * SW:(t + 1) * SW] = np.tile(flat.reshape(SW, 16).T, (8, 1))

    # per-node vectors in permuted order (pad rows -> 0)
    valid = perm >= 0
    def pv(vec):
        out = np.zeros(NP, np.float64)
        out[valid] = vec[perm[valid]]
        return out

    # s = A_hat @ 1, s2 = A_hat @ s (host, f64)
    def ahat(x):
        xp = dinv * x
        out = np.zeros(N, np.float64)
        np.add.at(out, dst, xp[src])
        return dinv * (out + xp)
    ones = np.ones(N, np.float64)
    s = ahat(ones)
    s2 = ahat(s)

    dv = pv(dinv)
    dinv1 = dv.reshape(T, P).T.astype(np.float32).copy()
    dinv2 = (dv ** 2).reshape(T, P).T.astype(np.float32).copy()

    def dev_slot(a):   # [T, C, P] -> [P, T*C]
        return np.ascontiguousarray(
            a.reshape(T, C, P).transpose(2, 0, 1).reshape(P, T * C))

    return dict(NP=NP, T=T, C=C, SW=SW, perm=perm, valid=valid,
                dinv=dv, deg=pv(deg), s=pv(s), s2=pv(s2),
                dinv1=dinv1, dinv2=dinv2, idx16=idx16, gsrc=gsrc,
                slot=dev_slot(slot).astype(BF))


def _build_nc(NP, T, C, SW):
    scratch = int(os.environ.get("KBASS_SCRATCH", "16384"))
    # HW num_idxs encode limit: 1024 passes, 1152+ crashes the Q7.
    # Gathers carry CH=8 128-slot chunks; tiles (C=6 chunks) straddle
    # gather boundaries.
    CH = int(os.environ.get("KBASS_CH", "8"))
    OHB = int(os.environ.get("KBASS_OHB", "10"))
    GB = int(os.environ.get("KBASS_GB", "10"))
    GBW = int(os.environ.get("KBASS_GBW", "7"))
    OHBW = int(os.environ.get("KBASS_OHBW", "10"))
    nc = bacc.Bacc("TRN2", dynamic_dma_scratch_size=scratch,
                   num_swdge_queues=4)
    NCHUNK = T * C
    SWC = P // 16      # idx16 cols per 128-slot chunk

    d = {}
    d["gA"] = nc.dram_tensor("gA", [P, T * C * NW], BF16, kind="ExternalInput")
    d["ohA"] = nc.dram_tensor("ohA", [P, T * C * P], BF16, kind="ExternalInput")
    d["t1sb"] = nc.dram_tensor("t1sb", [P, T * NW], BF16, kind="ExternalInput")
    d["rhsA"] = nc.dram_tensor("rhsA", [4, FM], BF16, kind="ExternalInput")
    d["rhsB"] = nc.dram_tensor("rhsB", [3, FM], BF16, kind="ExternalInput")
    d["cb34"] = nc.dram_tensor("cb34", [2, FM], BF16, kind="ExternalInput")
    d["cb56"] = nc.dram_tensor("cb56", [2, NW], BF16, kind="ExternalInput")
    d["sdrd"] = nc.dram_tensor("sdrd", [2, NP], BF16, kind="ExternalInput")
    d["s2s1"] = nc.dram_tensor("s2s1", [3, NP], BF16, kind="ExternalInput")
    d["W34"] = nc.dram_tensor("W34", [FM, FM], BF16, kind="ExternalInput")
    d["W56"] = nc.dram_tensor("W56", [FM, NW], BF16, kind="ExternalInput")
    d["idx16"] = nc.dram_tensor("idx16", [P, T * SW], I16, kind="ExternalInput")
    d["slotb"] = nc.dram_tensor("slotb", [P, T * C], BF16, kind="ExternalInput")
    d["dinv1"] = nc.dram_tensor("dinv1", [P, T], F32, kind="ExternalInput")
    d["dinv2"] = nc.dram_tensor("dinv2", [P, T], F32, kind="ExternalInput")
    out_d = nc.dram_tensor("out", [NP, NW], F32, kind="ExternalOutput")

    t2 = nc.dram_tensor("t2", [NP, FN], BF16, kind="Internal")
    t3 = nc.dram_tensor("t3", [NP, FM], BF16, kind="Internal")
    t4 = nc.dram_tensor("t4", [NP, FM], BF16, kind="Internal")
    t5 = nc.dram_tensor("t5", [NP, FN], BF16, kind="Internal")
    t6 = nc.dram_tensor("t6", [NP, FN], BF16, kind="Internal")

    Ident = mybir.ActivationFunctionType.Identity
    Relu = mybir.ActivationFunctionType.Relu

    with tile.TileContext(nc) as tc:
        with ExitStack() as ctx:
            res = ctx.enter_context(tc.tile_pool(name="res", bufs=1))
            idx_sb = res.tile([P, T * SW], I16)
            slot_sb = res.tile([P, T * C], BF16)
            dinv1_sb = res.tile([P, T], F32)
            dinv2_sb = res.tile([P, T], F32)
            hres1 = res.tile([P, T * NW], BF16)
            gA_sb = res.tile([P, T * C * NW], BF16)
            rhsA_sb = res.tile([4, FM], BF16)
            rhsB_sb = res.tile([3, FM], BF16)
            cb34_sb = res.tile([2, FM], BF16)
            cb56_sb = res.tile([2, NW], BF16)
            W34_sb = res.tile([P, KM * FM], BF16)
            W56_sb = res.tile([P, KM * NW], BF16)
            for name, t_sb in [("idx16", idx_sb), ("slotb", slot_sb),
                               ("dinv1", dinv1_sb), ("dinv2", dinv2_sb),
                               ("t1sb", hres1), ("gA", gA_sb),
                               ("rhsA", rhsA_sb),
                               ("rhsB", rhsB_sb), ("cb34", cb34_sb),
                               ("cb56", cb56_sb)]:
                nc.sync.dma_start(out=t_sb[:], in_=d[name][:, :])
            nc.sync.dma_start(
                out=W34_sb[:].rearrange("p (k f) -> p k f", k=KM),
                in_=d["W34"][:, :].rearrange("(k p) f -> p k f", p=P))
            nc.sync.dma_start(
                out=W56_sb[:].rearrange("p (k f) -> p k f", k=KM),
                in_=d["W56"][:, :].rearrange("(k p) f -> p k f", p=P))

            tail = NCHUNK % CH or CH
            reg_full = nc.gpsimd.to_reg(CH * P)
            reg_tail = nc.gpsimd.to_reg(tail * P)
            qctr = [0]
            iota_i = res.tile([P, P], mybir.dt.int32)
            nc.gpsimd.iota(iota_i[:], pattern=[[1, P]], base=0,
                           channel_multiplier=0)
            iota_b = res.tile([P, P], BF16)
            nc.vector.tensor_copy(out=iota_b[:], in_=iota_i[:])
            iota3 = res.tile([P, C * P], BF16)
            for c in range(C):
                nc.vector.tensor_copy(out=iota3[:, c * P:(c + 1) * P],
                                      in_=iota_b[:])
            ident_b = res.tile([P, P], BF16)
            make_identity(nc, ident_b[:])
            hres2 = res.tile([P, T * NW], BF16)
            hres5 = res.tile([P, T * NW], BF16)
            hres6 = res.tile([P, T * NW], BF16)
            outsb = res.tile([P, T * NW], F32)
            hresW = res.tile([P, T * FM], BF16)

            def make_stream(sp, src_d, width, tag):
                """Lazy chunk-stream gatherer: rhs(j) returns (g_tile, col
                offset) for global chunk j, issuing CH-chunk gathers."""
                bufs = {}

                def rhs(j):
                    b = j // CH
                    if b not in bufs:
                        n_ch = min(CH, NCHUNK - b * CH)
                        gt = sp.tile([P, CH * width], BF16, tag=tag,
                                     name=f"{tag}_{b}")
                        q = qctr[0] % 4
                        nc.gpsimd.dma_gather(
                            out_ap=gt[:, 0:n_ch * width]
                                .rearrange("p (q f) -> p q f", q=n_ch),
                            in_ap=src_d[:, :],
                            idxs_ap=idx_sb[:, b * CH * SWC:
                                           (b * CH + n_ch) * SWC],
                            num_idxs=n_ch * P,
                            num_idxs_reg=reg_full if n_ch == CH else reg_tail,
                            elem_size=width,
                            queue_num=q)
                        qctr[0] += 1
                        bufs[b] = gt
                    return bufs[b], (j - (j // CH) * CH) * width

                return rhs

            def load_oh(sp, t, tag):
                oh = sp.tile([P, C * P], BF16, tag=tag, name=f"{tag}_{t}")
                oh3 = oh[:].rearrange("p (c j) -> p c j", c=C)
                nc.vector.tensor_tensor(
                    out=oh3,
                    in0=slot_sb[:, t * C:(t + 1) * C]
                        .rearrange("p (c u) -> p c u", u=1).to_broadcast([P, C, P]),
                    in1=iota3[:].rearrange("p (c j) -> p c j", c=C),
                    op=mybir.AluOpType.is_equal,
                )
                return oh

            def narrow_write(dst_t, src_sb, gi, G):
                nc.sync.dma_start(
                    out=dst_t[gi * P:(gi + G) * P, 0:NW]
                        .rearrange("(t p) f -> p t f", p=P),
                    in_=src_sb[:, gi * NW:(gi + G) * NW]
                        .rearrange("p (t f) -> p t f", f=NW))

            # ---------------- phase A: t2 = dinv^2 (ADJ t1 + t1) ----------------
            with tc.tile_pool(name="pA", bufs=GB) as gp, \
                 tc.tile_pool(name="pAo", bufs=OHB) as op, \
                 tc.tile_pool(name="pAp", bufs=6, space="PSUM") as pp:
                for t in range(T):
                    oh = op.tile([P, C * P], BF16, tag="ohA", name=f"ohA_{t}")
                    nc.sync.dma_start(out=oh[:],
                                      in_=d["ohA"][:, t * C * P:(t + 1) * C * P])
                    pa = pp.tile([P, NW], F32, tag="pa", name=f"paA_{t}")
                    nc.tensor.matmul(out=pa[:], lhsT=ident_b[:],
                                     rhs=hres1[:, t * NW:(t + 1) * NW],
                                     start=True, stop=False)
                    for c in range(C):
                        j = t * C + c
                        nc.tensor.matmul(
                            out=pa[:], lhsT=oh[:, c * P:(c + 1) * P],
                            rhs=gA_sb[:, j * NW:(j + 1) * NW],
                            start=False, stop=(c == C - 1))
                    nc.vector.tensor_tensor(
                        out=hres2[:, t * NW:(t + 1) * NW], in0=pa[:],
                        in1=dinv2_sb[:, t:t + 1].to_broadcast([P, NW]),
                        op=mybir.AluOpType.mult)
                    if t % 9 == 8 or t == T - 1:
                        g0 = (t // 9) * 9
                        narrow_write(t2, hres2, g0, t - g0 + 1)

            # -------- phase B: VV = dinv(ADJ t2 + t2); x2T; relu; W34 -> t3 -----
            with tc.tile_pool(name="pB", bufs=GB) as gp, \
                 tc.tile_pool(name="pBo", bufs=OHB) as op, \
                 tc.tile_pool(name="pBs", bufs=4) as sp, \
                 tc.tile_pool(name="pBp", bufs=2, space="PSUM") as pp, \
                 tc.tile_pool(name="pBv", bufs=2, space="PSUM") as pv, \
                 tc.tile_pool(name="pBx", bufs=2, space="PSUM") as px, \
                 tc.tile_pool(name="pBc", bufs=1) as cp, \
                 tc.tile_pool(name="pBd", bufs=2, space="PSUM") as pd:
                s2s1_sb = cp.tile([3, NP], BF16)
                nc.sync.dma_start(out=s2s1_sb[:], in_=d["s2s1"][:, :])
                rhs = make_stream(gp, t2, FN, "gB")
                for t in range(T):
                    oh = op.tile([P, C * P], BF16, tag="ohB", name=f"ohB_{t}")
                    nc.sync.dma_start(out=oh[:],
                                      in_=d["ohA"][:, t * C * P:(t + 1) * C * P])
                    pa = pp.tile([P, NW], F32, tag="pa", name=f"paB_{t}")
                    nc.tensor.matmul(out=pa[:], lhsT=ident_b[:],
                                     rhs=hres2[:, t * NW:(t + 1) * NW],
                                     start=True, stop=False)
                    for c in range(C):
                        gt, off = rhs(t * C + c)
                        nc.tensor.matmul(
                            out=pa[:], lhsT=oh[:, c * P:(c + 1) * P],
                            rhs=gt[:, off:off + NW],
                            start=False, stop=(c == C - 1))
                    vvs = sp.tile([P, NW], BF16, tag="vvs", name=f"vv_{t}")
                    nc.scalar.activation(out=vvs[:], in_=pa[:], func=Ident,
                                         scale=dinv1_sb[:, t:t + 1])
                    ptv = pv.tile([NW, P], F32, tag="ptv", name=f"ptv_{t}")
                    nc.tensor.matmul(out=ptv[:], lhsT=vvs[:], rhs=ident_b[:],
                                     start=True, stop=True)
                    vvt = sp.tile([NW, P], BF16, tag="vvt", name=f"vvt_{t}")
                    nc.vector.tensor_copy(out=vvt[:], in_=ptv[:])
                    x2T = px.tile([P, FM], F32, tag="x2T", name=f"x2T_{t}")
                    for k in range(KM):
                        s_ = slice(k * P, (k + 1) * P)
                        nc.tensor.matmul(out=x2T[:, s_],
                                         lhsT=rhsA_sb[:, s_], rhs=vvt[:],
                                         start=True, stop=False)
                        nc.tensor.matmul(out=x2T[:, s_],
                                         lhsT=rhsB_sb[:, s_],
                                         rhs=s2s1_sb[:, t * P:(t + 1) * P],
                                         start=False, stop=True)
                    x2s = sp.tile([P, FM], BF16, tag="x2s", name=f"x2s_{t}")
                    nc.scalar.activation(out=x2s[:], in_=x2T[:], func=Relu)
                    pdn = pd.tile([P, FM], F32, tag="pdn", name=f"pdnB_{t}")
                    for k in range(KM):
                        nc.tensor.matmul(
                            out=pdn[:], lhsT=x2s[:, k * P:(k + 1) * P],
                            rhs=W34_sb[:, k * FM:(k + 1) * FM],
                            start=(k == 0), stop=(k == KM - 1))
                    nc.scalar.activation(out=hresW[:, t * FM:(t + 1) * FM],
                                         in_=pdn[:], func=Ident,
                                         scale=dinv1_sb[:, t:t + 1])
                    nc.sync.dma_start(out=t3[t * P:(t + 1) * P, :],
                                      in_=hresW[:, t * FM:(t + 1) * FM])

            # ---------------- phase C: t4 = dinv^2 (ADJ t3 + t3) ----------------
            with tc.tile_pool(name="pC", bufs=GBW) as gp, \
                 tc.tile_pool(name="pCo", bufs=OHBW) as op, \
                 tc.tile_pool(name="pCs", bufs=3) as sp, \
                 tc.tile_pool(name="pCp", bufs=4, space="PSUM") as pp:
                rhs = make_stream(gp, t3, FM, "gC")
                for t in range(T):
                    oh = load_oh(op, t, "ohC")
                    pa = pp.tile([P, FM], F32, tag="pa", name=f"paC_{t}")
                    for c in range(C):
                        gt, off = rhs(t * C + c)
                        nc.tensor.matmul(
                            out=pa[:], lhsT=oh[:, c * P:(c + 1) * P],
                            rhs=gt[:, off:off + FM],
                            start=(c == 0), stop=(c == C - 1))
                    # self-loop on DVE (pa + t3 rows), scale on ACT
                    nd = sp.tile([P, FM], BF16, tag="nd", name=f"ndC_{t}")
                    nc.vector.tensor_add(out=nd[:], in0=pa[:],
                                         in1=hresW[:, t * FM:(t + 1) * FM])
                    nc.scalar.activation(out=hresW[:, t * FM:(t + 1) * FM],
                                         in_=nd[:], func=Ident,
                                         scale=dinv2_sb[:, t:t + 1])
                    nc.sync.dma_start(out=t4[t * P:(t + 1) * P, :],
                                      in_=hresW[:, t * FM:(t + 1) * FM])

            # ---- phase D: x4T = (ADJ t4 + t4 + rank1)^T; relu; W56 -> t5 -------
            with tc.tile_pool(name="pD", bufs=GBW) as gp, \
                 tc.tile_pool(name="pDo", bufs=OHBW) as op, \
                 tc.tile_pool(name="pDs", bufs=3) as sp, \
                 tc.tile_pool(name="pDp", bufs=2, space="PSUM") as pp, \
                 tc.tile_pool(name="pDt", bufs=2, space="PSUM") as pt, \
                 tc.tile_pool(name="pDc", bufs=1) as cp, \
                 tc.tile_pool(name="pDd", bufs=2, space="PSUM") as pd:
                sdrd_sb = cp.tile([2, NP], BF16)
                nc.sync.dma_start(out=sdrd_sb[:], in_=d["sdrd"][:, :])
                rhs = make_stream(gp, t4, FM, "gD")
                for t in range(T):
                    oh = load_oh(op, t, "ohD")
                    pa = pp.tile([P, FM], F32, tag="pa", name=f"paD_{t}")
                    nc.tensor.matmul(out=pa[:],
                                     lhsT=sdrd_sb[:, t * P:(t + 1) * P],
                                     rhs=cb34_sb[:], start=True, stop=False)
                    nc.tensor.matmul(out=pa[:], lhsT=ident_b[:],
                                     rhs=hresW[:, t * FM:(t + 1) * FM],
                                     start=False, stop=False)
                    for c in range(C):
                        gt, off = rhs(t * C + c)
                        nc.tensor.matmul(
                            out=pa[:], lhsT=oh[:, c * P:(c + 1) * P],
                            rhs=gt[:, off:off + FM],
                            start=False, stop=(c == C - 1))
                    # z = relu(dinv * pa), node-major
                    zs = sp.tile([P, FM], BF16, tag="zs", name=f"zs_{t}")
                    nc.scalar.activation(out=zs[:], in_=pa[:], func=Relu,
                                         scale=dinv1_sb[:, t:t + 1])
                    ptr = pt.tile([P, FM], F32, tag="ptr", name=f"ptr_{t}")
                    for k in range(KM):
                        nc.tensor.matmul(
                            out=ptr[:, k * P:(k + 1) * P],
                            lhsT=zs[:, k * P:(k + 1) * P],
                            rhs=ident_b[:], start=True, stop=True)
                    zT = sp.tile([P, FM], BF16, tag="zT", name=f"zT_{t}")
                    nc.scalar.activation(out=zT[:], in_=ptr[:], func=Ident)
                    pdn = pd.tile([P, NW], F32, tag="pdn", name=f"pdnD_{t}")
                    for k in range(KM):
                        nc.tensor.matmul(
                            out=pdn[:], lhsT=zT[:, k * P:(k + 1) * P],
                            rhs=W56_sb[:, k * NW:(k + 1) * NW],
                            start=(k == 0), stop=(k == KM - 1))
                    nc.scalar.activation(out=hres5[:, t * NW:(t + 1) * NW],
                                         in_=pdn[:], func=Ident,
                                         scale=dinv1_sb[:, t:t + 1])
                    if t % 9 == 8 or t == T - 1:
                        g0 = (t // 9) * 9
                        narrow_write(t5, hres5, g0, t - g0 + 1)

            # ---------------- phase E: t6 = dinv^2 (ADJ t5 + t5) ----------------
            with tc.tile_pool(name="pE", bufs=GB) as gp, \
                 tc.tile_pool(name="pEo", bufs=OHB) as op, \
                 tc.tile_pool(name="pEp", bufs=6, space="PSUM") as pp:
                rhs = make_stream(gp, t5, FN, "gE")
                for t in range(T):
                    oh = op.tile([P, C * P], BF16, tag="ohE", name=f"ohE_{t}")
                    nc.sync.dma_start(out=oh[:],
                                      in_=d["ohA"][:, t * C * P:(t + 1) * C * P])
                    pa = pp.tile([P, NW], F32, tag="pa", name=f"paE_{t}")
                    nc.tensor.matmul(out=pa[:], lhsT=ident_b[:],
                                     rhs=hres5[:, t * NW:(t + 1) * NW],
                                     start=True, stop=False)
                    for c in range(C):
                        gt, off = rhs(t * C + c)
                        nc.tensor.matmul(
                            out=pa[:], lhsT=oh[:, c * P:(c + 1) * P],
                            rhs=gt[:, off:off + NW],
                            start=False, stop=(c == C - 1))
                    nc.scalar.activation(out=hres6[:, t * NW:(t + 1) * NW],
                                         in_=pa[:], func=Ident,
                                         scale=dinv2_sb[:, t:t + 1])
                    if t % 9 == 8 or t == T - 1:
                        g0 = (t // 9) * 9
                        narrow_write(t6, hres6, g0, t - g0 + 1)

            # ------- phase F: out = dinv (ADJ t6 + t6 + rank1(c5,b6)) -----------
            with tc.tile_pool(name="pF", bufs=GB) as gp, \
                 tc.tile_pool(name="pFo", bufs=OHB) as op, \
                 tc.tile_pool(name="pFc", bufs=1) as cp, \
                 tc.tile_pool(name="pFp", bufs=6, space="PSUM") as pp:
                sdrd_sb = cp.tile([2, NP], BF16)
                nc.sync.dma_start(out=sdrd_sb[:], in_=d["sdrd"][:, :])
                rhs = make_stream(gp, t6, FN, "gF")
                for t in range(T):
                    oh = op.tile([P, C * P], BF16, tag="ohF", name=f"ohF_{t}")
                    nc.sync.dma_start(out=oh[:],
                                      in_=d["ohA"][:, t * C * P:(t + 1) * C * P])
                    pa = pp.tile([P, NW], F32, tag="pa", name=f"paF_{t}")
                    nc.tensor.matmul(out=pa[:], lhsT=ident_b[:],
                                     rhs=hres6[:, t * NW:(t + 1) * NW],
                                     start=True, stop=False)
                    nc.tensor.matmul(out=pa[:],
                                     lhsT=sdrd_sb[:, t * P:(t + 1) * P],
                                     rhs=cb56_sb[:], start=False, stop=False)
                    for c in range(C):
                        gt, off = rhs(t * C + c)
                        nc.tensor.matmul(
                            out=pa[:], lhsT=oh[:, c * P:(c + 1) * P],
                            rhs=gt[:, off:off + NW],
                            start=False, stop=(c == C - 1))
                    nc.scalar.activation(out=outsb[:, t * NW:(t + 1) * NW],
                                         in_=pa[:], func=Ident,
                                         scale=dinv1_sb[:, t:t + 1])
                    if t % 9 == 8 or t == T - 1:
                        g0 = (t // 9) * 9
                        G = t - g0 + 1
                        nc.sync.dma_start(
                            out=out_d[g0 * P:(g0 + G) * P, :]
                                .rearrange("(t p) f -> p t f", p=P),
                            in_=outsb[:, g0 * NW:(g0 + G) * NW]
                                .rearrange("p (t f) -> p t f", f=NW))

    nc.compile()
    return nc


def _prepare(batch_vertices, img_features, edge_indices,
             W1, b1, W2, b2, W3, b3, W4, b4, W5, b5, W6, b6):
    B, N, _ = batch_vertices.shape
    ei = np.asarray(edge_indices).astype(np.int64)
    g = _pack_graph(ei[0], ei[1], N)
    NP, T, C, SW, perm, valid = (g["NP"], g["T"], g["C"], g["SW"],
                                 g["perm"], g["valid"])
    meta_T, meta_C = T, C

    W1f = np.asarray(W1, np.float64); W2f = np.asarray(W2, np.float64)
    W3f = np.asarray(W3, np.float64); W4f = np.asarray(W4, np.float64)
    W5f = np.asarray(W5, np.float64); W6f = np.asarray(W6, np.float64)
    W12 = W1f @ W2f
    W34 = W3f @ W4f
    W56 = W5f @ W6f
    c1 = np.asarray(b1, np.float64) @ W2f
    c3 = np.asarray(b3, np.float64) @ W4f
    c5 = np.asarray(b5, np.float64) @ W6f

    dinv, deg, s, s2 = g["dinv"], g["deg"], g["s"], g["s2"]
    sqdeg = np.sqrt(deg)

    # t1 = dinv * V (permuted, padded to FN cols)
    vp = np.zeros((B, NP, 3), np.float64)
    vp[:, valid, :] = np.asarray(batch_vertices, np.float64)[:, perm[valid], :]
    t1 = np.zeros((B, NP, FN), np.float64)
    t1[:, :, :3] = dinv[None, :, None] * vp

    rhsA = np.zeros((4, FM))
    rhsA[:3] = W12[:3]
    cb34 = np.stack([c3, np.asarray(b4, np.float64)])
    cb56 = np.zeros((2, NW))
    cb56[0, :3] = c5
    cb56[1, :3] = np.asarray(b6, np.float64)
    sdrd = np.stack([s * sqdeg, sqdeg])
    s2s1 = np.stack([s2, s, valid.astype(np.float64)])
    W56p = np.zeros((FM, NW))
    W56p[:, :3] = W56

    slot_dev = g["slot"].astype(np.float32)          # [P, T*C]
    ohA = (slot_dev[:, :, None] ==
           np.arange(P, dtype=np.float32)[None, None, :]).astype(BF)
    ohA = np.ascontiguousarray(ohA.reshape(slot_dev.shape[0], -1))
    common = {
        "ohA": ohA,
        "rhsA": rhsA.astype(BF), "cb34": cb34.astype(BF),
        "cb56": cb56.astype(BF), "sdrd": sdrd.astype(BF),
        "s2s1": s2s1.astype(BF), "W34": W34.astype(BF),
        "W56": W56p.astype(BF), "idx16": g["idx16"], "slotb": g["slot"],
        "dinv1": g["dinv1"], "dinv2": g["dinv2"],
    }
    gsrc = g["gsrc"]
    T_, C_ = meta_T, meta_C
    in_maps = []
    imgf = np.asarray(img_features, np.float64)
    for b in range(B):
        m = dict(common)
        gA = t1[b][gsrc.reshape(T_, C_, P), :NW]      # [T, C, P, NW]
        m["gA"] = np.ascontiguousarray(
            gA.transpose(2, 0, 1, 3).reshape(P, T_ * C_ * NW)).astype(BF)
        m["t1sb"] = np.ascontiguousarray(
            t1[b, :, :NW].reshape(T, P, NW).transpose(1, 0, 2)
            .reshape(P, T * NW)).astype(BF)
        rhsB = np.stack([imgf[b] @ W12[3:], c1, np.asarray(b2, np.float64)])
        m["rhsB"] = rhsB.astype(BF)
        in_maps.append(m)
    meta = dict(NP=NP, T=T, C=C, SW=SW, perm=perm, valid=valid, B=B, N=N)
    return in_maps, meta


_BUILD_CACHE = {}


def run(inputs, trace=False):
    in_maps, meta = _prepare(**inputs)
    key = (meta["NP"], meta["C"])
    if key not in _BUILD_CACHE:
        t0 = time.time()
        _BUILD_CACHE[key] = _build_nc(meta["NP"], meta["T"], meta["C"],
                                      meta["SW"])
        print(f"[kernel] built bass program in {time.time()-t0:.1f}s",
              file=sys.stderr)
    nc = _BUILD_CACHE[key]
    B = meta["B"]
    res = run_bass_kernel_spmd(nc, in_maps, core_ids=list(range(B)),
                               trace=trace)
    perm, valid, N = meta["perm"], meta["valid"], meta["N"]
    out = np.empty((B, N, 3), np.float32)
    for b in range(B):
        dev = res.results[b]["out"]
        out[b, perm[valid], :] = dev[valid, :3]
    return out, res


def kernel(**inputs) -> np.ndarray:
    out, _ = run(inputs)
    return out


# revision 52
# speedup vs baseline: 1.0142x; 1.0142x over previous
"""GCN (6-layer GCNConv) Trainium2 Bass kernel — v8.

Data-parallel over batch (1 mesh per NeuronCore).

Algebraic restructuring vs v6 (1.457 ms):
  - Layer pairs (1,2), (3,4), (5,6) have no ReLU inside, so
    A_hat(A_hat(x Wa) Wb) = A_hat^2(x) (Wa Wb) with host-premultiplied
    W12/W34/W56. Biases become rank-1 terms (A_hat 1 = s rows).
  - The img-feature half of the input is rank-1 across nodes:
    A_hat^2([V | 1 (x) f]) = [A_hat^2 V | s2 (x) f]. So phases A,B apply
    A_hat to WIDTH-3 vertex data; only C,D are 512-wide; E,F width-3.
  - Symmetric norm factorizes: tables prescaled by dinv, one-hot is pure
    0/1 (is_equal only, no norm multiply), post-scale dinv/dinv^2 folded
    into the PSUM-drain activation. Self-loop = identity matmul into the
    same PSUM accumulation (start=True slot).
  - Gathers carry 8 128-slot chunks (1024 idxs, the HW num_idxs encode
    limit; 1152+ crashes the Q7); tiles straddle gather boundaries.
    61 gathers/phase instead of 486 total amortizes the ~1us SWDGE fixed
    cost. Pool serializes each gather's desc-gen + transfer, so the whole
    kernel rides the Pool timeline (~1.1 ms floor at 256B/1KB rows).
  - Deep pools (10 gather/one-hot bufs narrow, 7 wide) keep the Pool fed;
    descriptor-ring scratch shrunk to 16KB to make SBUF room.
  - prepare_only+trigger_dma measured 3x WORSE here (8us/prep spans +
    1.4us InstIncSwdgeSem) - do not revisit without a runtime fix.
  - Phase A needs no device gathers at all: its gather source (dinv*V)
    and the one-hot tables are host-known, so the edge-ordered operands
    (gA) and phase-A one-hots (ohA) are uploaded pre-laid-out; phase A
    runs off Pool entirely (~60us saved).
"""
import sys
import time
import os

sys.path.insert(0, "/opt/trn_rl_repo")
import numpy as np
import ml_dtypes
from contextlib import ExitStack

import concourse.bass as bass
import concourse.bacc as bacc
import concourse.mybir as mybir
import concourse.tile as tile
from concourse.bass_utils import run_bass_kernel_spmd
from concourse.masks import make_identity

P = 128
F32 = mybir.dt.float32
BF16 = mybir.dt.bfloat16
I16 = mybir.dt.int16
BF = ml_dtypes.bfloat16

FM = 512   # hidden width
KM = FM // P
FN = 128   # narrow table row width in elems (256B min gather row)
NW = 4     # useful narrow cols (3 data + pad)


def _pack_graph(src, dst, N):
    """Relabel nodes into degree-balanced 128-node tiles (cap C*P edges per
    tile). slot[t, c*P+p] = dst offset in tile (or -1 pad), idx = src node."""
    T = (N + P - 1) // P
    NP = T * P
    E = len(src)
    indeg = np.bincount(dst, minlength=N)
    C = max(1, int(np.ceil(E / (T * P))))

    order = np.argsort(-indeg, kind="stable")
    while True:
        cap = C * P
        load = np.zeros(T, np.int64)
        count = np.zeros(T, np.int64)
        assign = np.empty(N, np.int64)
        ok = True
        for v in order:
            d = int(indeg[v])
            best_t, best_rem = -1, -1
            for t in range(T):
                if count[t] < P:
                    rem = cap - load[t]
                    if rem > best_rem:
                        best_rem, best_t = rem, t
            if best_t < 0 or load[best_t] + d > cap:
                ok = False
                break
            assign[v] = best_t
            load[best_t] += d
            count[best_t] += 1
        if ok:
            break
        C += 1

    perm = np.full(NP, -1, np.int64)
    new_of_old = np.empty(N, np.int64)
    cursor = np.zeros(T, np.int64)
    for v in range(N):
        t = assign[v]
        nid = t * P + cursor[t]
        cursor[t] += 1
        perm[nid] = v
        new_of_old[v] = nid

    deg = (indeg + 1).astype(np.float64)          # +1 self-loop (GCN)
    dinv = 1.0 / np.sqrt(deg)

    src_n = new_of_old[src]
    dst_n = new_of_old[dst]
    tile_of_e = dst_n // P
    order_e = np.argsort(tile_of_e, kind="stable")
    src_n, dst_n = src_n[order_e], dst_n[order_e]
    tile_of_e = tile_of_e[order_e]

    CP = C * P
    gsrc = np.zeros((T, CP), np.int32)
    slot = np.full((T, CP), -1.0, np.float32)
    starts = np.searchsorted(tile_of_e, np.arange(T + 1))
    for t in range(T):
        lo, hi = starts[t], starts[t + 1]
        n_e = hi - lo
        assert n_e <= CP, (t, n_e, CP)
        gsrc[t, :n_e] = src_n[lo:hi]
        slot[t, :n_e] = (dst_n[lo:hi] - t * P).astype(np.float32)

    # int16 index table: wrapped [i%16, i//16], replicated to all 8 stripes
    SW = CP // 16
    idx16 = np.zeros((P, T * SW), np.int16)
    for t in range(T):
        flat = gsrc[t].astype(np.int16)
        idx16[:, t * SW:(t + 1) * SW] = np.tile(flat.reshape(SW, 16).T, (8, 1))

    # per-node vectors in permuted order (pad rows -> 0)
    valid = perm >= 0
    def pv(vec):
        out = np.zeros(NP, np.float64)
        out[valid] = vec[perm[valid]]
        return out

    # s = A_hat @ 1, s2 = A_hat @ s (host, f64)
    def ahat(x):
        xp = dinv * x
        out = np.zeros(N, np.float64)
        np.add.at(out, dst, xp[src])
        return dinv * (out + xp)
    ones = np.ones(N, np.float64)
    s = ahat(ones)
    s2 = ahat(s)

    dv = pv(dinv)
    dinv1 = dv.reshape(T, P).T.astype(np.float32).copy()
    dinv2 = (dv ** 2).reshape(T, P).T.astype(np.float32).copy()

    def dev_slot(a):   # [T, C, P] -> [P, T*C]
        return np.ascontiguousarray(
            a.reshape(T, C, P).transpose(2, 0, 1).reshape(P, T * C))

    return dict(NP=NP, T=T, C=C, SW=SW, perm=perm, valid=valid,
                dinv=dv, deg=pv(deg), s=pv(s), s2=pv(s2),
                dinv1=dinv1, dinv2=dinv2, idx16=idx16, gsrc=gsrc,
                slot=dev_slot(slot).astype(BF))


def _build_nc(NP, T, C, SW):
    scratch = int(os.environ.get("KBASS_SCRATCH", "16384"))
    # HW num_idxs encode limit: 1024 passes, 1152+ crashes the Q7.
    # Gathers carry CH=8 128-slot chunks; tiles (C=6 chunks) straddle
    # gather boundaries.
    CH = int(os.environ.get("KBASS_CH", "8"))
    OHB = int(os.environ.get("KBASS_OHB", "10"))
    GB = int(os.environ.get("KBASS_GB", "10"))
    GBW = int(os.environ.get("KBASS_GBW", "7"))
    OHBW = int(os.environ.get("KBASS_OHBW", "10"))
    nc = bacc.Bacc("TRN2", dynamic_dma_scratch_size=scratch,
                   num_swdge_queues=4)
    NCHUNK = T * C
    SWC = P // 16      # idx16 cols per 128-slot chunk

    d = {}
    d["gA"] = nc.dram_tensor("gA", [P, T * C * NW], BF16, kind="ExternalInput")
    d["ohA"] = nc.dram_tensor("ohA", [P, T * C * P], BF16, kind="ExternalInput")
    d["t1sb"] = nc.dram_tensor("t1sb", [P, T * NW], BF16, kind="ExternalInput")
    d["rhsA"] = nc.dram_tensor("rhsA", [4, FM], BF16, kind="ExternalInput")
    d["rhsB"] = nc.dram_tensor("rhsB", [3, FM], BF16, kind="ExternalInput")
    d["cb34"] = nc.dram_tensor("cb34", [2, FM], BF16, kind="ExternalInput")
    d["cb56"] = nc.dram_tensor("cb56", [2, NW], BF16, kind="ExternalInput")
    d["sdrd"] = nc.dram_tensor("sdrd", [2, NP], BF16, kind="ExternalInput")
    d["s2s1"] = nc.dram_tensor("s2s1", [3, NP], BF16, kind="ExternalInput")
    d["W34"] = nc.dram_tensor("W34", [FM, FM], BF16, kind="ExternalInput")
    d["W56"] = nc.dram_tensor("W56", [FM, NW], BF16, kind="ExternalInput")
    d["idx16"] = nc.dram_tensor("idx16", [P, T * SW], I16, kind="ExternalInput")
    d["slotb"] = nc.dram_tensor("slotb", [P, T * C], BF16, kind="ExternalInput")
    d["dinv1"] = nc.dram_tensor("dinv1", [P, T], F32, kind="ExternalInput")
    d["dinv2"] = nc.dram_tensor("dinv2", [P, T], F32, kind="ExternalInput")
    out_d = nc.dram_tensor("out", [NP, NW], F32, kind="ExternalOutput")

    t2 = nc.dram_tensor("t2", [NP, FN], BF16, kind="Internal")
    t3 = nc.dram_tensor("t3", [NP, FM], BF16, kind="Internal")
    t4 = nc.dram_tensor("t4", [NP, FM], BF16, kind="Internal")
    t5 = nc.dram_tensor("t5", [NP, FN], BF16, kind="Internal")
    t6 = nc.dram_tensor("t6", [NP, FN], BF16, kind="Internal")

    Ident = mybir.ActivationFunctionType.Identity
    Relu = mybir.ActivationFunctionType.Relu

    with tile.TileContext(nc) as tc:
        with ExitStack() as ctx:
            res = ctx.enter_context(tc.tile_pool(name="res", bufs=1))
            idx_sb = res.tile([P, T * SW], I16)
            slot_sb = res.tile([P, T * C], BF16)
            dinv1_sb = res.tile([P, T], F32)
            dinv2_sb = res.tile([P, T], F32)
            hres1 = res.tile([P, T * NW], BF16)
            gA_sb = res.tile([P, T * C * NW], BF16)
            rhsA_sb = res.tile([4, FM], BF16)
            rhsB_sb = res.tile([3, FM], BF16)
            cb34_sb = res.tile([2, FM], BF16)
            cb56_sb = res.tile([2, NW], BF16)
            W34_sb = res.tile([P, KM * FM], BF16)
            W56_sb = res.tile([P, KM * NW], BF16)
            for name, t_sb in [("idx16", idx_sb), ("slotb", slot_sb),
                               ("dinv1", dinv1_sb), ("dinv2", dinv2_sb),
                               ("t1sb", hres1), ("gA", gA_sb),
                               ("rhsA", rhsA_sb),
                               ("rhsB", rhsB_sb), ("cb34", cb34_sb),
                               ("cb56", cb56_sb)]:
                nc.sync.dma_start(out=t_sb[:], in_=d[name][:, :])
            nc.sync.dma_start(
                out=W34_sb[:].rearrange("p (k f) -> p k f", k=KM),
                in_=d["W34"][:, :].rearrange("(k p) f -> p k f", p=P))
            nc.sync.dma_start(
                out=W56_sb[:].rearrange("p (k f) -> p k f", k=KM),
                in_=d["W56"][:, :].rearrange("(k p) f -> p k f", p=P))

            tail = NCHUNK % CH or CH
            reg_full = nc.gpsimd.to_reg(CH * P)
            reg_tail = nc.gpsimd.to_reg(tail * P)
            qctr = [0]
            iota_i = res.tile([P, P], mybir.dt.int32)
            nc.gpsimd.iota(iota_i[:], pattern=[[1, P]], base=0,
                           channel_multiplier=0)
            iota_b = res.tile([P, P], BF16)
            nc.vector.tensor_copy(out=iota_b[:], in_=iota_i[:])
            iota3 = res.tile([P, C * P], BF16)
            for c in range(C):
                nc.vector.tensor_copy(out=iota3[:, c * P:(c + 1) * P],
                                      in_=iota_b[:])
            ident_b = res.tile([P, P], BF16)
            make_identity(nc, ident_b[:])
            hres2 = res.tile([P, T * NW], BF16)
            hres5 = res.tile([P, T * NW], BF16)
            hres6 = res.tile([P, T * NW], BF16)
            outsb = res.tile([P, T * NW], F32)
            hresW = res.tile([P, T * FM], BF16)

            def make_stream(sp, src_d, width, tag):
                """Lazy chunk-stream gatherer: rhs(j) returns (g_tile, col
                offset) for global chunk j, issuing CH-chunk gathers."""
                bufs = {}

                def rhs(j):
                    b = j // CH
                    if b not in bufs:
                        n_ch = min(CH, NCHUNK - b * CH)
                        gt = sp.tile([P, CH * width], BF16, tag=tag,
                                     name=f"{tag}_{b}")
                        q = qctr[0] % 4
                        nc.gpsimd.dma_gather(
                            out_ap=gt[:, 0:n_ch * width]
                                .rearrange("p (q f) -> p q f", q=n_ch),
                            in_ap=src_d[:, :],
                            idxs_ap=idx_sb[:, b * CH * SWC:
                                           (b * CH + n_ch) * SWC],
                            num_idxs=n_ch * P,
                            num_idxs_reg=reg_full if n_ch == CH else reg_tail,
                            elem_size=width,
                            queue_num=q)
                        qctr[0] += 1
                        bufs[b] = gt
                    return bufs[b], (j - (j // CH) * CH) * width

                return rhs

            def load_oh(sp, t, tag):
                oh = sp.tile([P, C * P], BF16, tag=tag, name=f"{tag}_{t}")
                oh3 = oh[:].rearrange("p (c j) -> p c j", c=C)
                nc.vector.tensor_tensor(
                    out=oh3,
                    in0=slot_sb[:, t * C:(t + 1) * C]
                        .rearrange("p (c u) -> p c u", u=1).to_broadcast([P, C, P]),
                    in1=iota3[:].rearrange("p (c j) -> p c j", c=C),
                    op=mybir.AluOpType.is_equal,
                )
                return oh

            def narrow_write(dst_t, src_sb, gi, G):
                nc.sync.dma_start(
                    out=dst_t[gi * P:(gi + G) * P, 0:NW]
                        .rearrange("(t p) f -> p t f", p=P),
                    in_=src_sb[:, gi * NW:(gi + G) * NW]
                        .rearrange("p (t f) -> p t f", f=NW))

            # ---------------- phase A: t2 = dinv^2 (ADJ t1 + t1) ----------------
            with tc.tile_pool(name="pA", bufs=GB) as gp, \
                 tc.tile_pool(name="pAo", bufs=OHB) as op, \
                 tc.tile_pool(name="pAp", bufs=6, space="PSUM") as pp:
                for t in range(T):
                    oh = op.tile([P, C * P], BF16, tag="ohA", name=f"ohA_{t}")
                    nc.sync.dma_start(out=oh[:],
                                      in_=d["ohA"][:, t * C * P:(t + 1) * C * P])
                    pa = pp.tile([P, NW], F32, tag="pa", name=f"paA_{t}")
                    nc.tensor.matmul(out=pa[:], lhsT=ident_b[:],
                                     rhs=hres1[:, t * NW:(t + 1) * NW],
                                     start=True, stop=False)
                    for c in range(C):
                        j = t * C + c
                        nc.tensor.matmul(
                            out=pa[:], lhsT=oh[:, c * P:(c + 1) * P],
                            rhs=gA_sb[:, j * NW:(j + 1) * NW],
                            start=False, stop=(c == C - 1))
                    nc.vector.tensor_tensor(
                        out=hres2[:, t * NW:(t + 1) * NW], in0=pa[:],
                        in1=dinv2_sb[:, t:t + 1].to_broadcast([P, NW]),
                        op=mybir.AluOpType.mult)
                    if t % 9 == 8 or t == T - 1:
                        g0 = (t // 9) * 9
                        narrow_write(t2, hres2, g0, t - g0 + 1)

            # -------- phase B: VV = dinv(ADJ t2 + t2); x2T; relu; W34 -> t3 -----
            with tc.tile_pool(name="pB", bufs=GB) as gp, \
                 tc.tile_pool(name="pBo", bufs=OHB) as op, \
                 tc.tile_pool(name="pBs", bufs=4) as sp, \
                 tc.tile_pool(name="pBp", bufs=2, space="PSUM") as pp, \
                 tc.tile_pool(name="pBv", bufs=2, space="PSUM") as pv, \
                 tc.tile_pool(name="pBx", bufs=2, space="PSUM") as px, \
                 tc.tile_pool(name="pBc", bufs=1) as cp, \
                 tc.tile_pool(name="pBd", bufs=2, space="PSUM") as pd:
                s2s1_sb = cp.tile([3, NP], BF16)
                nc.sync.dma_start(out=s2s1_sb[:], in_=d["s2s1"][:, :])
                rhs = make_stream(gp, t2, FN, "gB")
                for t in range(T):
                    oh = load_oh(op, t, "ohB")
                    pa = pp.tile([P, NW], F32, tag="pa", name=f"paB_{t}")
                    nc.tensor.matmul(out=pa[:], lhsT=ident_b[:],
                                     rhs=hres2[:, t * NW:(t + 1) * NW],
                                     start=True, stop=False)
                    for c in range(C):
                        gt, off = rhs(t * C + c)
                        nc.tensor.matmul(
                            out=pa[:], lhsT=oh[:, c * P:(c + 1) * P],
                            rhs=gt[:, off:off + NW],
                            start=False, stop=(c == C - 1))
                    vvs = sp.tile([P, NW], BF16, tag="vvs", name=f"vv_{t}")
                    nc.scalar.activation(out=vvs[:], in_=pa[:], func=Ident,
                                         scale=dinv1_sb[:, t:t + 1])
                    ptv = pv.tile([NW, P], F32, tag="ptv", name=f"ptv_{t}")
                    nc.tensor.matmul(out=ptv[:], lhsT=vvs[:], rhs=ident_b[:],
                                     start=True, stop=True)
                    vvt = sp.tile([NW, P], BF16, tag="vvt", name=f"vvt_{t}")
                    nc.vector.tensor_copy(out=vvt[:], in_=ptv[:])
                    x2T = px.tile([P, FM], F32, tag="x2T", name=f"x2T_{t}")
                    for k in range(KM):
                        s_ = slice(k * P, (k + 1) * P)
                        nc.tensor.matmul(out=x2T[:, s_],
                                         lhsT=rhsA_sb[:, s_], rhs=vvt[:],
                                         start=True, stop=False)
                        nc.tensor.matmul(out=x2T[:, s_],
                                         lhsT=rhsB_sb[:, s_],
                                         rhs=s2s1_sb[:, t * P:(t + 1) * P],
                                         start=False, stop=True)
                    x2s = sp.tile([P, FM], BF16, tag="x2s", name=f"x2s_{t}")
                    nc.scalar.activation(out=x2s[:], in_=x2T[:], func=Relu)
                    pdn = pd.tile([P, FM], F32, tag="pdn", name=f"pdnB_{t}")
                    for k in range(KM):
                        nc.tensor.matmul(
                            out=pdn[:], lhsT=x2s[:, k * P:(k + 1) * P],
                            rhs=W34_sb[:, k * FM:(k + 1) * FM],
                            start=(k == 0), stop=(k == KM - 1))
                    nc.scalar.activation(out=hresW[:, t * FM:(t + 1) * FM],
                                         in_=pdn[:], func=Ident,
                                         scale=dinv1_sb[:, t:t + 1])
                    nc.sync.dma_start(out=t3[t * P:(t + 1) * P, :],
                                      in_=hresW[:, t * FM:(t + 1) * FM])

            # ---------------- phase C: t4 = dinv^2 (ADJ t3 + t3) ----------------
            with tc.tile_pool(name="pC", bufs=GBW) as gp, \
                 tc.tile_pool(name="pCo", bufs=OHBW) as op, \
                 tc.tile_pool(name="pCs", bufs=3) as sp, \
                 tc.tile_pool(name="pCp", bufs=4, space="PSUM") as pp:
                rhs = make_stream(gp, t3, FM, "gC")
                for t in range(T):
                    oh = load_oh(op, t, "ohC")
                    pa = pp.tile([P, FM], F32, tag="pa", name=f"paC_{t}")
                    for c in range(C):
                        gt, off = rhs(t * C + c)
                        nc.tensor.matmul(
                            out=pa[:], lhsT=oh[:, c * P:(c + 1) * P],
                            rhs=gt[:, off:off + FM],
                            start=(c == 0), stop=(c == C - 1))
                    # self-loop on DVE (pa + t3 rows), scale on ACT
                    nd = sp.tile([P, FM], BF16, tag="nd", name=f"ndC_{t}")
                    nc.vector.tensor_add(out=nd[:], in0=pa[:],
                                         in1=hresW[:, t * FM:(t + 1) * FM])
                    nc.scalar.activation(out=hresW[:, t * FM:(t + 1) * FM],
                                         in_=nd[:], func=Ident,
                                         scale=dinv2_sb[:, t:t + 1])
                    nc.sync.dma_start(out=t4[t * P:(t + 1) * P, :],
                                      in_=hresW[:, t * FM:(t + 1) * FM])

            # ---- phase D: x4T = (ADJ t4 + t4 + rank1)^T; relu; W56 -> t5 -------
            with tc.tile_pool(name="pD", bufs=GBW) as gp, \
                 tc.tile_pool(name="pDo", bufs=OHBW) as op, \
                 tc.tile_pool(name="pDs", bufs=3) as sp, \
                 tc.tile_pool(name="pDp", bufs=2, space="PSUM") as pp, \
                 tc.tile_pool(name="pDt", bufs=2, space="PSUM") as pt, \
                 tc.tile_pool(name="pDc", bufs=1) as cp, \
                 tc.tile_pool(name="pDd", bufs=2, space="PSUM") as pd:
                sdrd_sb = cp.tile([2, NP], BF16)
                nc.sync.dma_start(out=sdrd_sb[:], in_=d["sdrd"][:, :])
                rhs = make_stream(gp, t4, FM, "gD")
                for t in range(T):
                    oh = load_oh(op, t, "ohD")
                    pa = pp.tile([P, FM], F32, tag="pa", name=f"paD_{t}")
                    nc.tensor.matmul(out=pa[:],
                                     lhsT=sdrd_sb[:, t * P:(t + 1) * P],
                                     rhs=cb34_sb[:], start=True, stop=False)
                    nc.tensor.matmul(out=pa[:], lhsT=ident_b[:],
                                     rhs=hresW[:, t * FM:(t + 1) * FM],
                                     start=False, stop=False)
                    for c in range(C):
                        gt, off = rhs(t * C + c)
                        nc.tensor.matmul(
                            out=pa[:], lhsT=oh[:, c * P:(c + 1) * P],
                            rhs=gt[:, off:off + FM],
                            start=False, stop=(c == C - 1))
                    # z = relu(dinv * pa), node-major
                    zs = sp.tile([P, FM], BF16, tag="zs", name=f"zs_{t}")
                    nc.scalar.activation(out=zs[:], in_=pa[:], func=Relu,
                                         scale=dinv1_sb[:, t:t + 1])
                    ptr = pt.tile([P, FM], F32, tag="ptr", name=f"ptr_{t}")
                    for k in range(KM):
                        nc.tensor.matmul(
                            out=ptr[:, k * P:(k + 1) * P],
                            lhsT=zs[:, k * P:(k + 1) * P],
                            rhs=ident_b[:], start=True, stop=True)
                    zT = sp.tile([P, FM], BF16, tag="zT", name=f"zT_{t}")
                    nc.scalar.activation(out=zT[:], in_=ptr[:], func=Ident)
                    pdn = pd.tile([P, NW], F32, tag="pdn", name=f"pdnD_{t}")
                    for k in range(KM):
                        nc.tensor.matmul(
                            out=pdn[:], lhsT=zT[:, k * P:(k + 1) * P],
                            rhs=W56_sb[:, k * NW:(k + 1) * NW],
                            start=(k == 0), stop=(k == KM - 1))
                    nc.scalar.activation(out=hres5[:, t * NW:(t + 1) * NW],
                                         in_=pdn[:], func=Ident,
                                         scale=dinv1_sb[:, t:t + 1])
                    if t % 9 == 8 or t == T - 1:
                        g0 = (t // 9) * 9
                        narrow_write(t5, hres5, g0, t - g0 + 1)

            # ---------------- phase E: t6 = dinv^2 (ADJ t5 + t5) ----------------
            with tc.tile_pool(name="pE", bufs=GB) as gp, \
                 tc.tile_pool(name="pEo", bufs=OHB) as op, \
                 tc.tile_pool(name="pEp", bufs=6, space="PSUM") as pp:
                rhs = make_stream(gp, t5, FN, "gE")
                for t in range(T):
                    oh = load_oh(op, t, "ohE")
                    pa = pp.tile([P, NW], F32, tag="pa", name=f"paE_{t}")
                    nc.tensor.matmul(out=pa[:], lhsT=ident_b[:],
                                     rhs=hres5[:, t * NW:(t + 1) * NW],
                                     start=True, stop=False)
                    for c in range(C):
                        gt, off = rhs(t * C + c)
                        nc.tensor.matmul(
                            out=pa[:], lhsT=oh[:, c * P:(c + 1) * P],
                            rhs=gt[:, off:off + NW],
                            start=False, stop=(c == C - 1))
                    nc.scalar.activation(out=hres6[:, t * NW:(t + 1) * NW],
                                         in_=pa[:], func=Ident,
                                         scale=dinv2_sb[:, t:t + 1])
                    if t % 9 == 8 or t == T - 1:
                        g0 = (t // 9) * 9
                        narrow_write(t6, hres6, g0, t - g0 + 1)

            # ------- phase F: out = dinv (ADJ t6 + t6 + rank1(c5,b6)) -----------
            with tc.tile_pool(name="pF", bufs=GB) as gp, \
                 tc.tile_pool(name="pFo", bufs=OHB) as op, \
                 tc.tile_pool(name="pFc", bufs=1) as cp, \
                 tc.tile_pool(name="pFp", bufs=6, space="PSUM") as pp:
                sdrd_sb = cp.tile([2, NP], BF16)
                nc.sync.dma_start(out=sdrd_sb[:], in_=d["sdrd"][:, :])
                rhs = make_stream(gp, t6, FN, "gF")
                for t in range(T):
                    oh = load_oh(op, t, "ohF")
                    pa = pp.tile([P, NW], F32, tag="pa", name=f"paF_{t}")
                    nc.tensor.matmul(out=pa[:], lhsT=ident_b[:],
                                     rhs=hres6[:, t * NW:(t + 1) * NW],
                                     start=True, stop=False)
                    nc.tensor.matmul(out=pa[:],
                                     lhsT=sdrd_sb[:, t * P:(t + 1) * P],
                                     rhs=cb56_sb[:], start=False, stop=False)
                    for c in range(C):
                        gt, off = rhs(t * C + c)
                        nc.tensor.matmul(
                            out=pa[:], lhsT=oh[:, c * P:(c + 1) * P],
                            rhs=gt[:, off:off + NW],
                            start=False, stop=(c == C - 1))
                    nc.scalar.activation(out=outsb[:, t * NW:(t + 1) * NW],
                                         in_=pa[:], func=Ident,
                                         scale=dinv1_sb[:, t:t + 1])
                    if t % 9 == 8 or t == T - 1:
                        g0 = (t // 9) * 9
                        G = t - g0 + 1
                        nc.sync.dma_start(
                            out=out_d[g0 * P:(g0 + G) * P, :]
                                .rearrange("(t p) f -> p t f", p=P),
                            in_=outsb[:, g0 * NW:(g0 + G) * NW]
                                .rearrange("p (t f) -> p t f", f=NW))

    nc.compile()
    return nc


def _prepare(batch_vertices, img_features, edge_indices,
             W1, b1, W2, b2, W3, b3, W4, b4, W5, b5, W6, b6):
    B, N, _ = batch_vertices.shape
    ei = np.asarray(edge_indices).astype(np.int64)
    g = _pack_graph(ei[0], ei[1], N)
    NP, T, C, SW, perm, valid = (g["NP"], g["T"], g["C"], g["SW"],
                                 g["perm"], g["valid"])
    meta_T, meta_C = T, C

    W1f = np.asarray(W1, np.float64); W2f = np.asarray(W2, np.float64)
    W3f = np.asarray(W3, np.float64); W4f = np.asarray(W4, np.float64)
    W5f = np.asarray(W5, np.float64); W6f = np.asarray(W6, np.float64)
    W12 = W1f @ W2f
    W34 = W3f @ W4f
    W56 = W5f @ W6f
    c1 = np.asarray(b1, np.float64) @ W2f
    c3 = np.asarray(b3, np.float64) @ W4f
    c5 = np.asarray(b5, np.float64) @ W6f

    dinv, deg, s, s2 = g["dinv"], g["deg"], g["s"], g["s2"]
    sqdeg = np.sqrt(deg)

    # t1 = dinv * V (permuted, padded to FN cols)
    vp = np.zeros((B, NP, 3), np.float64)
    vp[:, valid, :] = np.asarray(batch_vertices, np.float64)[:, perm[valid], :]
    t1 = np.zeros((B, NP, FN), np.float64)
    t1[:, :, :3] = dinv[None, :, None] * vp

    rhsA = np.zeros((4, FM))
    rhsA[:3] = W12[:3]
    cb34 = np.stack([c3, np.asarray(b4, np.float64)])
    cb56 = np.zeros((2, NW))
    cb56[0, :3] = c5
    cb56[1, :3] = np.asarray(b6, np.float64)
    sdrd = np.stack([s * sqdeg, sqdeg])
    s2s1 = np.stack([s2, s, valid.astype(np.float64)])
    W56p = np.zeros((FM, NW))
    W56p[:, :3] = W56

    slot_dev = g["slot"].astype(np.float32)          # [P, T*C]
    ohA = (slot_dev[:, :, None] ==
           np.arange(P, dtype=np.float32)[None, None, :]).astype(BF)
    ohA = np.ascontiguousarray(ohA.reshape(slot_dev.shape[0], -1))
    common = {
        "ohA": ohA,
        "rhsA": rhsA.astype(BF), "cb34": cb34.astype(BF),
        "cb56": cb56.astype(BF), "sdrd": sdrd.astype(BF),
        "s2s1": s2s1.astype(BF), "W34": W34.astype(BF),
        "W56": W56p.astype(BF), "idx16": g["idx16"], "slotb": g["slot"],
        "dinv1": g["dinv1"], "dinv2": g["dinv2"],
    }
    gsrc = g["gsrc"]
    T_, C_ = meta_T, meta_C
    in_maps = []
    imgf = np.asarray(img_features, np.float64)
    for b in range(B):
        m = dict(common)
        gA = t1[b][gsrc.reshape(T_, C_, P), :NW]      # [T, C, P, NW]
        m["gA"] = np.ascontiguousarray(
            gA.transpose(2, 0, 1, 3).reshape(P, T_ * C_ * NW)).astype(BF)
        m["t1sb"] = np.ascontiguousarray(
            t1[b, :, :NW].reshape(T, P, NW).transpose(1, 0, 2)
            .reshape(P, T * NW)).astype(BF)
        rhsB = np.stack([imgf[b] @ W12[3:], c1, np.asarray(b2, np.float64)])
        m["rhsB"] = rhsB.astype(BF)
        in_maps.append(m)
    meta = dict(NP=NP, T=T, C=C, SW=SW, perm=perm, valid=valid, B=B, N=N)
    return in_maps, meta


_BUILD_CACHE = {}


def run(inputs, trace=False):
    in_maps, meta = _prepare(**inputs)
    key = (meta["NP"], meta["C"])
    if key not in _BUILD_CACHE:
        t0 = time.time()
        _BUILD_CACHE[key] = _build_nc(meta["NP"], meta["T"], meta["C"],
                                      meta["SW"])
        print(f"[kernel] built bass program in {time.time()-t0:.1f}s",
              file=sys.stderr)
    nc = _BUILD_CACHE[key]
    B = meta["B"]
    res = run_bass_kernel_spmd(nc, in_maps, core_ids=list(range(B)),
                               trace=trace)
    perm, valid, N = meta["perm"], meta["valid"], meta["N"]
    out = np.empty((B, N, 3), np.float32)
    for b in range(B):
        dev = res.results[b]["out"]
        out[b, perm[valid], :] = dev[valid, :3]
    return out, res


def kernel(**inputs) -> np.ndarray:
    out, _ = run(inputs)
    return out


# revision 53
# speedup vs baseline: 1.1588x; 1.1425x over previous
"""GCN (6-layer GCNConv) Trainium2 Bass kernel — v8.

Data-parallel over batch (1 mesh per NeuronCore).

Algebraic restructuring vs v6 (1.457 ms):
  - Layer pairs (1,2), (3,4), (5,6) have no ReLU inside, so
    A_hat(A_hat(x Wa) Wb) = A_hat^2(x) (Wa Wb) with host-premultiplied
    W12/W34/W56. Biases become rank-1 terms (A_hat 1 = s rows).
  - The img-feature half of the input is rank-1 across nodes:
    A_hat^2([V | 1 (x) f]) = [A_hat^2 V | s2 (x) f]. So phases A,B apply
    A_hat to WIDTH-3 vertex data; only C,D are 512-wide; E,F width-3.
  - Symmetric norm factorizes: tables prescaled by dinv, one-hot is pure
    0/1 (is_equal only, no norm multiply), post-scale dinv/dinv^2 folded
    into the PSUM-drain activation. Self-loop = identity matmul into the
    same PSUM accumulation (start=True slot).
  - Gathers carry 8 128-slot chunks (1024 idxs, the HW num_idxs encode
    limit; 1152+ crashes the Q7); tiles straddle gather boundaries.
    61 gathers/phase instead of 486 total amortizes the ~1us SWDGE fixed
    cost. Pool serializes each gather's desc-gen + transfer, so the whole
    kernel rides the Pool timeline (~1.1 ms floor at 256B/1KB rows).
  - Deep pools (10 gather/one-hot bufs narrow, 7 wide) keep the Pool fed;
    descriptor-ring scratch shrunk to 16KB to make SBUF room.
  - prepare_only+trigger_dma measured 3x WORSE here (8us/prep spans +
    1.4us InstIncSwdgeSem) - do not revisit without a runtime fix.
  - Phase A needs no device gathers at all: its gather source (dinv*V)
    and the one-hot tables are host-known, so the edge-ordered operands
    (gA) and phase-A one-hots (ohA) are uploaded pre-laid-out; phase A
    runs off Pool entirely (~60us saved).
"""
import sys
import time
import os

sys.path.insert(0, "/opt/trn_rl_repo")
import numpy as np
import ml_dtypes
from contextlib import ExitStack

import concourse.bass as bass
import concourse.bacc as bacc
import concourse.mybir as mybir
import concourse.tile as tile
from concourse.bass_utils import run_bass_kernel_spmd
from concourse.masks import make_identity

P = 128
F32 = mybir.dt.float32
BF16 = mybir.dt.bfloat16
I16 = mybir.dt.int16
BF = ml_dtypes.bfloat16

FM = 512   # hidden width
KM = FM // P
FN = 128   # narrow table row width in elems (256B min gather row)
NW = 4     # useful narrow cols (3 data + pad)


def _pack_graph(src, dst, N):
    """Relabel nodes into degree-balanced 128-node tiles (cap C*P edges per
    tile). slot[t, c*P+p] = dst offset in tile (or -1 pad), idx = src node."""
    T = (N + P - 1) // P
    NP = T * P
    E = len(src)
    indeg = np.bincount(dst, minlength=N)
    C = max(1, int(np.ceil(E / (T * P))))

    order = np.argsort(-indeg, kind="stable")
    while True:
        cap = C * P
        load = np.zeros(T, np.int64)
        count = np.zeros(T, np.int64)
        assign = np.empty(N, np.int64)
        ok = True
        for v in order:
            d = int(indeg[v])
            best_t, best_rem = -1, -1
            for t in range(T):
                if count[t] < P:
                    rem = cap - load[t]
                    if rem > best_rem:
                        best_rem, best_t = rem, t
            if best_t < 0 or load[best_t] + d > cap:
                ok = False
                break
            assign[v] = best_t
            load[best_t] += d
            count[best_t] += 1
        if ok:
            break
        C += 1

    perm = np.full(NP, -1, np.int64)
    new_of_old = np.empty(N, np.int64)
    cursor = np.zeros(T, np.int64)
    for v in range(N):
        t = assign[v]
        nid = t * P + cursor[t]
        cursor[t] += 1
        perm[nid] = v
        new_of_old[v] = nid

    deg = (indeg + 1).astype(np.float64)          # +1 self-loop (GCN)
    dinv = 1.0 / np.sqrt(deg)

    src_n = new_of_old[src]
    dst_n = new_of_old[dst]
    tile_of_e = dst_n // P
    order_e = np.argsort(tile_of_e, kind="stable")
    src_n, dst_n = src_n[order_e], dst_n[order_e]
    tile_of_e = tile_of_e[order_e]

    CP = C * P
    gsrc = np.zeros((T, CP), np.int32)
    slot = np.full((T, CP), -1.0, np.float32)
    starts = np.searchsorted(tile_of_e, np.arange(T + 1))
    for t in range(T):
        lo, hi = starts[t], starts[t + 1]
        n_e = hi - lo
        assert n_e <= CP, (t, n_e, CP)
        gsrc[t, :n_e] = src_n[lo:hi]
        slot[t, :n_e] = (dst_n[lo:hi] - t * P).astype(np.float32)

    # int16 index table: wrapped [i%16, i//16], replicated to all 8 stripes
    SW = CP // 16
    idx16 = np.zeros((P, T * SW), np.int16)
    for t in range(T):
        flat = gsrc[t].astype(np.int16)
        idx16[:, t * SW:(t + 1) * SW] = np.tile(flat.reshape(SW, 16).T, (8, 1))

    # per-node vectors in permuted order (pad rows -> 0)
    valid = perm >= 0
    def pv(vec):
        out = np.zeros(NP, np.float64)
        out[valid] = vec[perm[valid]]
        return out

    # s = A_hat @ 1, s2 = A_hat @ s (host, f64)
    def ahat(x):
        xp = dinv * x
        out = np.zeros(N, np.float64)
        np.add.at(out, dst, xp[src])
        return dinv * (out + xp)
    ones = np.ones(N, np.float64)
    s = ahat(ones)
    s2 = ahat(s)

    dv = pv(dinv)
    dinv1 = dv.reshape(T, P).T.astype(np.float32).copy()
    dinv2 = (dv ** 2).reshape(T, P).T.astype(np.float32).copy()

    def dev_slot(a):   # [T, C, P] -> [P, T*C]
        return np.ascontiguousarray(
            a.reshape(T, C, P).transpose(2, 0, 1).reshape(P, T * C))

    return dict(NP=NP, T=T, C=C, SW=SW, perm=perm, valid=valid,
                dinv=dv, deg=pv(deg), s=pv(s), s2=pv(s2),
                dinv1=dinv1, dinv2=dinv2, idx16=idx16, gsrc=gsrc,
                slot=dev_slot(slot).astype(BF))


def _build_nc(NP, T, C, SW):
    scratch = int(os.environ.get("KBASS_SCRATCH", "16384"))
    # HW num_idxs encode limit: 1024 passes, 1152+ crashes the Q7.
    # Gathers carry CH=8 128-slot chunks; tiles (C=6 chunks) straddle
    # gather boundaries.
    CH = int(os.environ.get("KBASS_CH", "8"))
    OHB = int(os.environ.get("KBASS_OHB", "10"))
    GB = int(os.environ.get("KBASS_GB", "10"))
    GBW = int(os.environ.get("KBASS_GBW", "7"))
    OHBW = int(os.environ.get("KBASS_OHBW", "10"))
    nc = bacc.Bacc("TRN2", dynamic_dma_scratch_size=scratch,
                   num_swdge_queues=4)
    NCHUNK = T * C
    SWC = P // 16      # idx16 cols per 128-slot chunk

    d = {}
    d["gA"] = nc.dram_tensor("gA", [P, T * C * NW], BF16, kind="ExternalInput")
    d["ohA"] = nc.dram_tensor("ohA", [P, T * C * P], BF16, kind="ExternalInput")
    d["t1sb"] = nc.dram_tensor("t1sb", [P, T * NW], BF16, kind="ExternalInput")
    d["rhsA"] = nc.dram_tensor("rhsA", [4, FM], BF16, kind="ExternalInput")
    d["rhsB"] = nc.dram_tensor("rhsB", [3, FM], BF16, kind="ExternalInput")
    d["cb34"] = nc.dram_tensor("cb34", [2, FM], BF16, kind="ExternalInput")
    d["cb56"] = nc.dram_tensor("cb56", [2, NW], BF16, kind="ExternalInput")
    d["sdrd"] = nc.dram_tensor("sdrd", [2, NP], BF16, kind="ExternalInput")
    d["s2s1"] = nc.dram_tensor("s2s1", [3, NP], BF16, kind="ExternalInput")
    d["W34"] = nc.dram_tensor("W34", [FM, FM], BF16, kind="ExternalInput")
    d["W56"] = nc.dram_tensor("W56", [FM, NW], BF16, kind="ExternalInput")
    d["idx16"] = nc.dram_tensor("idx16", [P, T * SW], I16, kind="ExternalInput")
    d["slotb"] = nc.dram_tensor("slotb", [P, T * C], BF16, kind="ExternalInput")
    d["dinv1"] = nc.dram_tensor("dinv1", [P, T], F32, kind="ExternalInput")
    d["dinv2"] = nc.dram_tensor("dinv2", [P, T], F32, kind="ExternalInput")
    out_d = nc.dram_tensor("out", [NP, NW], F32, kind="ExternalOutput")

    t2 = nc.dram_tensor("t2", [NP, FN], BF16, kind="Internal")
    t3 = nc.dram_tensor("t3", [NP, FM], BF16, kind="Internal")
    t4 = nc.dram_tensor("t4", [NP, FM], BF16, kind="Internal")
    t5 = nc.dram_tensor("t5", [NP, FN], BF16, kind="Internal")
    t6 = nc.dram_tensor("t6", [NP, FN], BF16, kind="Internal")

    Ident = mybir.ActivationFunctionType.Identity
    Relu = mybir.ActivationFunctionType.Relu

    with tile.TileContext(nc) as tc:
        with ExitStack() as ctx:
            res = ctx.enter_context(tc.tile_pool(name="res", bufs=1))
            idx_sb = res.tile([P, T * SW], I16)
            slot_sb = res.tile([P, T * C], BF16)
            dinv1_sb = res.tile([P, T], F32)
            dinv2_sb = res.tile([P, T], F32)
            hres1 = res.tile([P, T * NW], BF16)
            gA_sb = res.tile([P, T * C * NW], BF16)
            rhsA_sb = res.tile([4, FM], BF16)
            rhsB_sb = res.tile([3, FM], BF16)
            cb34_sb = res.tile([2, FM], BF16)
            cb56_sb = res.tile([2, NW], BF16)
            W34_sb = res.tile([P, KM * FM], BF16)
            W56_sb = res.tile([P, KM * NW], BF16)
            for name, t_sb in [("idx16", idx_sb), ("slotb", slot_sb),
                               ("dinv1", dinv1_sb), ("dinv2", dinv2_sb),
                               ("t1sb", hres1), ("gA", gA_sb),
                               ("rhsA", rhsA_sb),
                               ("rhsB", rhsB_sb), ("cb34", cb34_sb),
                               ("cb56", cb56_sb)]:
                nc.sync.dma_start(out=t_sb[:], in_=d[name][:, :])
            nc.sync.dma_start(
                out=W34_sb[:].rearrange("p (k f) -> p k f", k=KM),
                in_=d["W34"][:, :].rearrange("(k p) f -> p k f", p=P))
            nc.sync.dma_start(
                out=W56_sb[:].rearrange("p (k f) -> p k f", k=KM),
                in_=d["W56"][:, :].rearrange("(k p) f -> p k f", p=P))

            tail = NCHUNK % CH or CH
            reg_full = nc.gpsimd.to_reg(CH * P)
            reg_tail = nc.gpsimd.to_reg(tail * P)
            qctr = [0]
            iota_i = res.tile([P, P], mybir.dt.int32)
            nc.gpsimd.iota(iota_i[:], pattern=[[1, P]], base=0,
                           channel_multiplier=0)
            iota_b = res.tile([P, P], BF16)
            nc.vector.tensor_copy(out=iota_b[:], in_=iota_i[:])
            iota3 = res.tile([P, C * P], BF16)
            for c in range(C):
                nc.vector.tensor_copy(out=iota3[:, c * P:(c + 1) * P],
                                      in_=iota_b[:])
            ident_b = res.tile([P, P], BF16)
            make_identity(nc, ident_b[:])
            hres2 = res.tile([P, T * NW], BF16)
            hres5 = res.tile([P, T * NW], BF16)
            hres6 = res.tile([P, T * NW], BF16)
            outsb = res.tile([P, T * NW], F32)
            hresW = res.tile([P, T * FM], BF16)

            def make_stream(sp, src_d, width, tag):
                """Lazy chunk-stream gatherer: rhs(j) returns (g_tile, col
                offset) for global chunk j, issuing CH-chunk gathers."""
                bufs = {}

                def rhs(j):
                    b = j // CH
                    if b not in bufs:
                        n_ch = min(CH, NCHUNK - b * CH)
                        gt = sp.tile([P, CH * width], BF16, tag=tag,
                                     name=f"{tag}_{b}")
                        q = qctr[0] % 4
                        nc.gpsimd.dma_gather(
                            out_ap=gt[:, 0:n_ch * width]
                                .rearrange("p (q f) -> p q f", q=n_ch),
                            in_ap=src_d[:, :],
                            idxs_ap=idx_sb[:, b * CH * SWC:
                                           (b * CH + n_ch) * SWC],
                            num_idxs=n_ch * P,
                            num_idxs_reg=reg_full if n_ch == CH else reg_tail,
                            elem_size=width,
                            queue_num=q)
                        qctr[0] += 1
                        bufs[b] = gt
                    return bufs[b], (j - (j // CH) * CH) * width

                return rhs

            def load_oh(sp, t, tag):
                oh = sp.tile([P, C * P], BF16, tag=tag, name=f"{tag}_{t}")
                oh3 = oh[:].rearrange("p (c j) -> p c j", c=C)
                nc.vector.tensor_tensor(
                    out=oh3,
                    in0=slot_sb[:, t * C:(t + 1) * C]
                        .rearrange("p (c u) -> p c u", u=1).to_broadcast([P, C, P]),
                    in1=iota3[:].rearrange("p (c j) -> p c j", c=C),
                    op=mybir.AluOpType.is_equal,
                )
                return oh

            def narrow_write(dst_t, src_sb, gi, G):
                nc.sync.dma_start(
                    out=dst_t[gi * P:(gi + G) * P, 0:NW]
                        .rearrange("(t p) f -> p t f", p=P),
                    in_=src_sb[:, gi * NW:(gi + G) * NW]
                        .rearrange("p (t f) -> p t f", f=NW))

            # ---------------- phase A: t2 = dinv^2 (ADJ t1 + t1) ----------------
            with tc.tile_pool(name="pA", bufs=GB) as gp, \
                 tc.tile_pool(name="pAo", bufs=OHB) as op, \
                 tc.tile_pool(name="pAp", bufs=6, space="PSUM") as pp:
                for t in range(T):
                    oh = op.tile([P, C * P], BF16, tag="ohA", name=f"ohA_{t}")
                    nc.sync.dma_start(out=oh[:],
                                      in_=d["ohA"][:, t * C * P:(t + 1) * C * P])
                    pa = pp.tile([P, NW], F32, tag="pa", name=f"paA_{t}")
                    nc.tensor.matmul(out=pa[:], lhsT=ident_b[:],
                                     rhs=hres1[:, t * NW:(t + 1) * NW],
                                     start=True, stop=False)
                    for c in range(C):
                        j = t * C + c
                        nc.tensor.matmul(
                            out=pa[:], lhsT=oh[:, c * P:(c + 1) * P],
                            rhs=gA_sb[:, j * NW:(j + 1) * NW],
                            start=False, stop=(c == C - 1))
                    nc.vector.tensor_tensor(
                        out=hres2[:, t * NW:(t + 1) * NW], in0=pa[:],
                        in1=dinv2_sb[:, t:t + 1].to_broadcast([P, NW]),
                        op=mybir.AluOpType.mult)
                    if t % 9 == 8 or t == T - 1:
                        g0 = (t // 9) * 9
                        narrow_write(t2, hres2, g0, t - g0 + 1)

            # -------- phase B: VV = dinv(ADJ t2 + t2); x2T; relu; W34 -> t3 -----
            with tc.tile_pool(name="pB", bufs=GB) as gp, \
                 tc.tile_pool(name="pBo", bufs=OHB) as op, \
                 tc.tile_pool(name="pBs", bufs=4) as sp, \
                 tc.tile_pool(name="pBp", bufs=2, space="PSUM") as pp, \
                 tc.tile_pool(name="pBv", bufs=2, space="PSUM") as pv, \
                 tc.tile_pool(name="pBx", bufs=2, space="PSUM") as px, \
                 tc.tile_pool(name="pBc", bufs=1) as cp, \
                 tc.tile_pool(name="pBd", bufs=2, space="PSUM") as pd:
                s2s1_sb = cp.tile([3, NP], BF16)
                nc.sync.dma_start(out=s2s1_sb[:], in_=d["s2s1"][:, :])
                rhs = make_stream(gp, t2, FN, "gB")
                for t in range(T):
                    oh = load_oh(op, t, "ohB")
                    pa = pp.tile([P, NW], F32, tag="pa", name=f"paB_{t}")
                    nc.tensor.matmul(out=pa[:], lhsT=ident_b[:],
                                     rhs=hres2[:, t * NW:(t + 1) * NW],
                                     start=True, stop=False)
                    for c in range(C):
                        gt, off = rhs(t * C + c)
                        nc.tensor.matmul(
                            out=pa[:], lhsT=oh[:, c * P:(c + 1) * P],
                            rhs=gt[:, off:off + NW],
                            start=False, stop=(c == C - 1))
                    vvs = sp.tile([P, NW], BF16, tag="vvs", name=f"vv_{t}")
                    nc.scalar.activation(out=vvs[:], in_=pa[:], func=Ident,
                                         scale=dinv1_sb[:, t:t + 1])
                    ptv = pv.tile([NW, P], F32, tag="ptv", name=f"ptv_{t}")
                    nc.tensor.matmul(out=ptv[:], lhsT=vvs[:], rhs=ident_b[:],
                                     start=True, stop=True)
                    vvt = sp.tile([NW, P], BF16, tag="vvt", name=f"vvt_{t}")
                    nc.vector.tensor_copy(out=vvt[:], in_=ptv[:])
                    x2T = px.tile([P, FM], F32, tag="x2T", name=f"x2T_{t}")
                    for k in range(KM):
                        s_ = slice(k * P, (k + 1) * P)
                        nc.tensor.matmul(out=x2T[:, s_],
                                         lhsT=rhsA_sb[:, s_], rhs=vvt[:],
                                         start=True, stop=False)
                        nc.tensor.matmul(out=x2T[:, s_],
                                         lhsT=rhsB_sb[:, s_],
                                         rhs=s2s1_sb[:, t * P:(t + 1) * P],
                                         start=False, stop=True)
                    x2s = sp.tile([P, FM], BF16, tag="x2s", name=f"x2s_{t}")
                    nc.scalar.activation(out=x2s[:], in_=x2T[:], func=Relu)
                    pdn = pd.tile([P, FM], F32, tag="pdn", name=f"pdnB_{t}")
                    for k in range(KM):
                        nc.tensor.matmul(
                            out=pdn[:], lhsT=x2s[:, k * P:(k + 1) * P],
                            rhs=W34_sb[:, k * FM:(k + 1) * FM],
                            start=(k == 0), stop=(k == KM - 1))
                    nc.scalar.activation(out=hresW[:, t * FM:(t + 1) * FM],
                                         in_=pdn[:], func=Ident,
                                         scale=dinv1_sb[:, t:t + 1])
                    nc.sync.dma_start(out=t3[t * P:(t + 1) * P, :],
                                      in_=hresW[:, t * FM:(t + 1) * FM])

            # ---------------- phase C: t4 = dinv^2 (ADJ t3 + t3) ----------------
            with tc.tile_pool(name="pC", bufs=GBW) as gp, \
                 tc.tile_pool(name="pCo", bufs=OHBW) as op, \
                 tc.tile_pool(name="pCs", bufs=3) as sp, \
                 tc.tile_pool(name="pCp", bufs=4, space="PSUM") as pp:
                rhs = make_stream(gp, t3, FM, "gC")
                for t in range(T):
                    oh = load_oh(op, t, "ohC")
                    pa = pp.tile([P, FM], F32, tag="pa", name=f"paC_{t}")
                    nc.tensor.matmul(out=pa[:], lhsT=ident_b[:],
                                     rhs=hresW[:, t * FM:(t + 1) * FM],
                                     start=True, stop=False)
                    for c in range(C):
                        gt, off = rhs(t * C + c)
                        nc.tensor.matmul(
                            out=pa[:], lhsT=oh[:, c * P:(c + 1) * P],
                            rhs=gt[:, off:off + FM],
                            start=False, stop=(c == C - 1))
                    nc.scalar.activation(out=hresW[:, t * FM:(t + 1) * FM],
                                         in_=pa[:], func=Ident,
                                         scale=dinv2_sb[:, t:t + 1])
                    nc.sync.dma_start(out=t4[t * P:(t + 1) * P, :],
                                      in_=hresW[:, t * FM:(t + 1) * FM])

            # ---- phase D: x4T = (ADJ t4 + t4 + rank1)^T; relu; W56 -> t5 -------
            with tc.tile_pool(name="pD", bufs=GBW) as gp, \
                 tc.tile_pool(name="pDo", bufs=OHBW) as op, \
                 tc.tile_pool(name="pDs", bufs=3) as sp, \
                 tc.tile_pool(name="pDp", bufs=2, space="PSUM") as pp, \
                 tc.tile_pool(name="pDt", bufs=2, space="PSUM") as pt, \
                 tc.tile_pool(name="pDc", bufs=1) as cp, \
                 tc.tile_pool(name="pDd", bufs=2, space="PSUM") as pd:
                sdrd_sb = cp.tile([2, NP], BF16)
                nc.sync.dma_start(out=sdrd_sb[:], in_=d["sdrd"][:, :])
                rhs = make_stream(gp, t4, FM, "gD")
                for t in range(T):
                    oh = load_oh(op, t, "ohD")
                    pa = pp.tile([P, FM], F32, tag="pa", name=f"paD_{t}")
                    nc.tensor.matmul(out=pa[:],
                                     lhsT=sdrd_sb[:, t * P:(t + 1) * P],
                                     rhs=cb34_sb[:], start=True, stop=False)
                    nc.tensor.matmul(out=pa[:], lhsT=ident_b[:],
                                     rhs=hresW[:, t * FM:(t + 1) * FM],
                                     start=False, stop=False)
                    for c in range(C):
                        gt, off = rhs(t * C + c)
                        nc.tensor.matmul(
                            out=pa[:], lhsT=oh[:, c * P:(c + 1) * P],
                            rhs=gt[:, off:off + FM],
                            start=False, stop=(c == C - 1))
                    # z = relu(dinv * pa), node-major
                    zs = sp.tile([P, FM], BF16, tag="zs", name=f"zs_{t}")
                    nc.scalar.activation(out=zs[:], in_=pa[:], func=Relu,
                                         scale=dinv1_sb[:, t:t + 1])
                    ptr = pt.tile([P, FM], F32, tag="ptr", name=f"ptr_{t}")
                    for k in range(KM):
                        nc.tensor.matmul(
                            out=ptr[:, k * P:(k + 1) * P],
                            lhsT=zs[:, k * P:(k + 1) * P],
                            rhs=ident_b[:], start=True, stop=True)
                    zT = sp.tile([P, FM], BF16, tag="zT", name=f"zT_{t}")
                    nc.scalar.activation(out=zT[:], in_=ptr[:], func=Ident)
                    pdn = pd.tile([P, NW], F32, tag="pdn", name=f"pdnD_{t}")
                    for k in range(KM):
                        nc.tensor.matmul(
                            out=pdn[:], lhsT=zT[:, k * P:(k + 1) * P],
                            rhs=W56_sb[:, k * NW:(k + 1) * NW],
                            start=(k == 0), stop=(k == KM - 1))
                    nc.scalar.activation(out=hres5[:, t * NW:(t + 1) * NW],
                                         in_=pdn[:], func=Ident,
                                         scale=dinv1_sb[:, t:t + 1])
                    if t % 9 == 8 or t == T - 1:
                        g0 = (t // 9) * 9
                        narrow_write(t5, hres5, g0, t - g0 + 1)

            # ---------------- phase E: t6 = dinv^2 (ADJ t5 + t5) ----------------
            with tc.tile_pool(name="pE", bufs=GB) as gp, \
                 tc.tile_pool(name="pEo", bufs=OHB) as op, \
                 tc.tile_pool(name="pEp", bufs=6, space="PSUM") as pp:
                rhs = make_stream(gp, t5, FN, "gE")
                for t in range(T):
                    oh = load_oh(op, t, "ohE")
                    pa = pp.tile([P, NW], F32, tag="pa", name=f"paE_{t}")
                    nc.tensor.matmul(out=pa[:], lhsT=ident_b[:],
                                     rhs=hres5[:, t * NW:(t + 1) * NW],
                                     start=True, stop=False)
                    for c in range(C):
                        gt, off = rhs(t * C + c)
                        nc.tensor.matmul(
                            out=pa[:], lhsT=oh[:, c * P:(c + 1) * P],
                            rhs=gt[:, off:off + NW],
                            start=False, stop=(c == C - 1))
                    nc.scalar.activation(out=hres6[:, t * NW:(t + 1) * NW],
                                         in_=pa[:], func=Ident,
                                         scale=dinv2_sb[:, t:t + 1])
                    if t % 9 == 8 or t == T - 1:
                        g0 = (t // 9) * 9
                        narrow_write(t6, hres6, g0, t - g0 + 1)

            # ------- phase F: out = dinv (ADJ t6 + t6 + rank1(c5,b6)) -----------
            with tc.tile_pool(name="pF", bufs=GB) as gp, \
                 tc.tile_pool(name="pFo", bufs=OHB) as op, \
                 tc.tile_pool(name="pFc", bufs=1) as cp, \
                 tc.tile_pool(name="pFp", bufs=6, space="PSUM") as pp:
                sdrd_sb = cp.tile([2, NP], BF16)
                nc.sync.dma_start(out=sdrd_sb[:], in_=d["sdrd"][:, :])
                rhs = make_stream(gp, t6, FN, "gF")
                for t in range(T):
                    oh = load_oh(op, t, "ohF")
                    pa = pp.tile([P, NW], F32, tag="pa", name=f"paF_{t}")
                    nc.tensor.matmul(out=pa[:], lhsT=ident_b[:],
                                     rhs=hres6[:, t * NW:(t + 1) * NW],
                                     start=True, stop=False)
                    nc.tensor.matmul(out=pa[:],
                                     lhsT=sdrd_sb[:, t * P:(t + 1) * P],
                                     rhs=cb56_sb[:], start=False, stop=False)
                    for c in range(C):
                        gt, off = rhs(t * C + c)
                        nc.tensor.matmul(
                            out=pa[:], lhsT=oh[:, c * P:(c + 1) * P],
                            rhs=gt[:, off:off + NW],
                            start=False, stop=(c == C - 1))
                    nc.scalar.activation(out=outsb[:, t * NW:(t + 1) * NW],
                                         in_=pa[:], func=Ident,
                                         scale=dinv1_sb[:, t:t + 1])
                    if t % 9 == 8 or t == T - 1:
                        g0 = (t // 9) * 9
                        G = t - g0 + 1
                        nc.sync.dma_start(
                            out=out_d[g0 * P:(g0 + G) * P, :]
                                .rearrange("(t p) f -> p t f", p=P),
                            in_=outsb[:, g0 * NW:(g0 + G) * NW]
                                .rearrange("p (t f) -> p t f", f=NW))

    nc.compile()
    return nc


def _prepare(batch_vertices, img_features, edge_indices,
             W1, b1, W2, b2, W3, b3, W4, b4, W5, b5, W6, b6):
    B, N, _ = batch_vertices.shape
    ei = np.asarray(edge_indices).astype(np.int64)
    g = _pack_graph(ei[0], ei[1], N)
    NP, T, C, SW, perm, valid = (g["NP"], g["T"], g["C"], g["SW"],
                                 g["perm"], g["valid"])
    meta_T, meta_C = T, C

    W1f = np.asarray(W1, np.float64); W2f = np.asarray(W2, np.float64)
    W3f = np.asarray(W3, np.float64); W4f = np.asarray(W4, np.float64)
    W5f = np.asarray(W5, np.float64); W6f = np.asarray(W6, np.float64)
    W12 = W1f @ W2f
    W34 = W3f @ W4f
    W56 = W5f @ W6f
    c1 = np.asarray(b1, np.float64) @ W2f
    c3 = np.asarray(b3, np.float64) @ W4f
    c5 = np.asarray(b5, np.float64) @ W6f

    dinv, deg, s, s2 = g["dinv"], g["deg"], g["s"], g["s2"]
    sqdeg = np.sqrt(deg)

    # t1 = dinv * V (permuted, padded to FN cols)
    vp = np.zeros((B, NP, 3), np.float64)
    vp[:, valid, :] = np.asarray(batch_vertices, np.float64)[:, perm[valid], :]
    t1 = np.zeros((B, NP, FN), np.float64)
    t1[:, :, :3] = dinv[None, :, None] * vp

    rhsA = np.zeros((4, FM))
    rhsA[:3] = W12[:3]
    cb34 = np.stack([c3, np.asarray(b4, np.float64)])
    cb56 = np.zeros((2, NW))
    cb56[0, :3] = c5
    cb56[1, :3] = np.asarray(b6, np.float64)
    sdrd = np.stack([s * sqdeg, sqdeg])
    s2s1 = np.stack([s2, s, valid.astype(np.float64)])
    W56p = np.zeros((FM, NW))
    W56p[:, :3] = W56

    slot_dev = g["slot"].astype(np.float32)          # [P, T*C]
    ohA = (slot_dev[:, :, None] ==
           np.arange(P, dtype=np.float32)[None, None, :]).astype(BF)
    ohA = np.ascontiguousarray(ohA.reshape(slot_dev.shape[0], -1))
    common = {
        "ohA": ohA,
        "rhsA": rhsA.astype(BF), "cb34": cb34.astype(BF),
        "cb56": cb56.astype(BF), "sdrd": sdrd.astype(BF),
        "s2s1": s2s1.astype(BF), "W34": W34.astype(BF),
        "W56": W56p.astype(BF), "idx16": g["idx16"], "slotb": g["slot"],
        "dinv1": g["dinv1"], "dinv2": g["dinv2"],
    }
    gsrc = g["gsrc"]
    T_, C_ = meta_T, meta_C
    in_maps = []
    imgf = np.asarray(img_features, np.float64)
    for b in range(B):
        m = dict(common)
        gA = t1[b][gsrc.reshape(T_, C_, P), :NW]      # [T, C, P, NW]
        m["gA"] = np.ascontiguousarray(
            gA.transpose(2, 0, 1, 3).reshape(P, T_ * C_ * NW)).astype(BF)
        m["t1sb"] = np.ascontiguousarray(
            t1[b, :, :NW].reshape(T, P, NW).transpose(1, 0, 2)
            .reshape(P, T * NW)).astype(BF)
        rhsB = np.stack([imgf[b] @ W12[3:], c1, np.asarray(b2, np.float64)])
        m["rhsB"] = rhsB.astype(BF)
        in_maps.append(m)
    meta = dict(NP=NP, T=T, C=C, SW=SW, perm=perm, valid=valid, B=B, N=N)
    return in_maps, meta


_BUILD_CACHE = {}


def run(inputs, trace=False):
    in_maps, meta = _prepare(**inputs)
    key = (meta["NP"], meta["C"])
    if key not in _BUILD_CACHE:
        t0 = time.time()
        _BUILD_CACHE[key] = _build_nc(meta["NP"], meta["T"], meta["C"],
                                      meta["SW"])
        print(f"[kernel] built bass program in {time.time()-t0:.1f}s",
              file=sys.stderr)
    nc = _BUILD_CACHE[key]
    B = meta["B"]
    res = run_bass_kernel_spmd(nc, in_maps, core_ids=list(range(B)),
                               trace=trace)
    perm, valid, N = meta["perm"], meta["valid"], meta["N"]
    out = np.empty((B, N, 3), np.float32)
    for b in range(B):
        dev = res.results[b]["out"]
        out[b, perm[valid], :] = dev[valid, :3]
    return out, res


def kernel(**inputs) -> np.ndarray:
    out, _ = run(inputs)
    return out
